# revision 30
# baseline (speedup 1.0000x reference)
"""Trainium2 Bass kernel for nn_Attention_76854144795156.

Computes, per batch b:
    h   = tanh(a[b] @ W1a + s_prev[b] @ W1s + b1)      # [T, 10]
    e   = relu(h @ W2 + b2)                            # [T, 1]
    sco = softmax(e, axis=0)                           # [T, 1]
    ctx = sco.T @ a[b]                                 # [1, D]
Returns (context_vector [B,1,D], attention_score [B,T,1]).

Sharding: pure data parallel over batch across 8 NeuronCores
(8 batches per core); the tiny Dense weights are replicated.

The first matmul needs a^T (contraction over the feature dim), which on
the PE requires transposing `a`, and fp32 matmuls/weight-loads run at
half rate (LOW_HIGH 2-pass emulation) with non-overlappable weight
loads.  The kernel therefore streams `a` in bf16 (cast inline by the
DMA on the HBM->SBUF load; accumulation stays fp32 in PSUM), while the
softmax chain and the s_prev contribution are computed in fp32.
"""

import os
import numpy as np
from contextlib import ExitStack

B, T, D, S = 64, 512, 1024, 1024
NCORES = 8
BC = B // NCORES  # batches per core
NTB = T // 128    # t-blocks per batch (4)
NC1 = D // 128    # d-chunks for W1a (8)
NC2 = S // 128    # s-chunks for W1s (8)

MODE = os.environ.get("ATT_KERNEL_MODE", "bf16_all")  # fp32 | bf16_mm1 | bf16_all

_CACHE = {}


def _build(mode):
    import concourse.tile as tile
    from concourse import bacc, mybir
    from concourse.masks import make_identity

    f32 = mybir.dt.float32
    bf16 = mybir.dt.bfloat16
    AF = mybir.ActivationFunctionType
    AX = mybir.AxisListType
    ALU = mybir.AluOpType
    mm1_bf = mode != "fp32"
    mm2_bf = mode == "bf16_all"

    nc = bacc.Bacc("TRN2", num_devices=NCORES)

    a_d = nc.dram_tensor("a", [BC, T, D], f32, kind="ExternalInput")
    s_d = nc.dram_tensor("s_prev", [BC, S], f32, kind="ExternalInput")
    w1_d = nc.dram_tensor("W1", [D + S, 10], f32, kind="ExternalInput")
    b1_d = nc.dram_tensor("b1", [10], f32, kind="ExternalInput")
    w2_d = nc.dram_tensor("W2", [10, 1], f32, kind="ExternalInput")
    b2_d = nc.dram_tensor("b2", [1], f32, kind="ExternalInput")
    ctx_d = nc.dram_tensor("ctx_out", [BC, D], f32, kind="ExternalOutput")
    sco_d = nc.dram_tensor("score_out", [BC, T], f32, kind="ExternalOutput")

    with tile.TileContext(nc) as tc, ExitStack() as ctx:
        const_pool = ctx.enter_context(tc.tile_pool(name="const", bufs=1))
        a_pool = ctx.enter_context(
            tc.tile_pool(name="a_res", bufs=(8 if mm2_bf else 4 * BC))
        )
        abf_pool = ctx.enter_context(
            tc.tile_pool(name="a_bf", bufs=(4 * BC if mm2_bf else 8))
        )
        aT_pool = ctx.enter_context(tc.tile_pool(name="aT", bufs=10))
        row_pool = ctx.enter_context(tc.tile_pool(name="rows", bufs=3))
        ps_aT = ctx.enter_context(tc.tile_pool(name="ps_aT", bufs=2, space="PSUM"))
        ps_h = ctx.enter_context(tc.tile_pool(name="ps_h", bufs=2, space="PSUM"))
        ps_e = ctx.enter_context(tc.tile_pool(name="ps_e", bufs=1, space="PSUM"))
        ps_sT = ctx.enter_context(tc.tile_pool(name="ps_sT", bufs=1, space="PSUM"))
        ps_ctx = ctx.enter_context(tc.tile_pool(name="ps_ctx", bufs=1, space="PSUM"))

        # ---- issue the big streaming loads first: SWDGE starts filling
        # SBUF while the identity/constant preamble runs on other engines ----
        tdt = bf16 if mm1_bf else f32
        sdt = bf16 if mm2_bf else f32
        # ---- identity: bulk zero on DVE, diagonal fill on GpSimd (the
        # only gpsimd preamble op, so the SWDGE loads issue right after) ----
        def _make_identity(ap):
            nc.vector.memset(ap, 0.0)
            nc.gpsimd.affine_select(
                out=ap,
                in_=ap,
                compare_op=mybir.AluOpType.not_equal,
                fill=1.0,
                base=0,
                pattern=[[-1, ap.shape[0]]],
                channel_multiplier=1,
            )

        ident8 = const_pool.tile([BC, BC], sdt, tag="ident8")
        _make_identity(ident8[:])
        if mm1_bf:
            ident_t = const_pool.tile([128, 128], bf16, tag="ident_bf")
            _make_identity(ident_t[:])
            ident = ident_t if mm2_bf else const_pool.tile([128, 128], f32, tag="ident")
            if not mm2_bf:
                _make_identity(ident[:])
        else:
            ident = const_pool.tile([128, 128], f32, tag="ident")
            _make_identity(ident[:])
            ident_t = ident

        # ---- big streaming loads (SWDGE casts fp32->bf16 inline) ----
        all_src = {}
        s_sb = const_pool.tile([BC, S], sdt, tag="s")
        if mm2_bf:
            nc.gpsimd.dma_start(out=s_sb[:], in_=s_d[:, :])
            for b in range(BC):
                for tb in range(NTB):
                    a_bf = abf_pool.tile([128, D], bf16, tag="abf", name=f"abf{b}_{tb}")
                    nc.gpsimd.dma_start(
                        out=a_bf[:], in_=a_d[b, 128 * tb : 128 * (tb + 1), :]
                    )
                    all_src[(b, tb)] = a_bf
        else:
            nc.sync.dma_start(out=s_sb[:], in_=s_d[:, :])
        ones11 = const_pool.tile([1, 1], sdt, tag="ones11")
        nc.vector.memset(ones11[:], 1.0)

        w1_sb = const_pool.tile([128, 160], f32, tag="w1")  # 16 chunks of 10 cols
        nc.sync.dma_start(
            out=w1_sb.rearrange("p (c u) -> p c u", u=10),
            in_=w1_d.rearrange("(c p) u -> p c u", p=128),
        )
        if mm1_bf:
            w1_t = const_pool.tile([128, 160], bf16, tag="w1_bf")
            nc.vector.tensor_copy(w1_t[:], w1_sb[:])
        else:
            w1_t = w1_sb
        w2_sb = const_pool.tile([10, 1], f32, tag="w2")
        nc.sync.dma_start(out=w2_sb[:], in_=w2_d[:, :])
        if mm2_bf:
            w2_t = const_pool.tile([10, 1], bf16, tag="w2_bf")
            nc.vector.tensor_copy(w2_t[:], w2_sb[:])
        else:
            w2_t = w2_sb
        b1_sb = const_pool.tile([10, 1], f32, tag="b1")
        nc.sync.dma_start(out=b1_sb[:], in_=b1_d.rearrange("(u o) -> u o", o=1))
        b2_sb = const_pool.tile([1, 1], f32, tag="b2")
        nc.sync.dma_start(out=b2_sb[:], in_=b2_d.rearrange("(u o) -> u o", o=1))

        # ---- s_contrib = W1s.T @ s_prev.T -> [10, BC] ----
        sT_sb = const_pool.tile([128, NC2 * BC], sdt, tag="sT")
        for c in range(NC2):
            sT_ps = ps_sT.tile([128, BC], f32, tag="sT")
            nc.tensor.matmul(
                sT_ps[:],
                lhsT=s_sb[:, 128 * c : 128 * (c + 1)],
                rhs=ident8[:],
                start=True,
                stop=True,
            )
            nc.vector.tensor_copy(sT_sb[:, BC * c : BC * (c + 1)], sT_ps[:])
        sc_ps = ps_h.tile([10, BC], f32, tag="h")
        for c in range(NC2):
            nc.tensor.matmul(
                sc_ps[:],
                lhsT=(w1_t if mm2_bf else w1_sb)[
                    :, 10 * (NC1 + c) : 10 * (NC1 + c) + 10
                ],
                rhs=sT_sb[:, BC * c : BC * (c + 1)],
                start=(c == 0),
                stop=(c == NC2 - 1),
            )
        bias_sb = const_pool.tile([10, BC], f32, tag="bias")
        nc.scalar.activation(bias_sb[:], sc_ps[:], AF.Identity, bias=b1_sb[:])

        # ---- per-batch main loop ----
        for b in range(BC):
            if mm2_bf:
                # loaded (with inline fp32->bf16 cast) up front
                src_tiles = [all_src[(b, tb)] for tb in range(NTB)]
            else:
                src_tiles = []
                a_tiles = []
                for tb in range(NTB):
                    a_t = a_pool.tile([128, D], f32, tag="a")
                    nc.sync.dma_start(
                        out=a_t[:], in_=a_d[b, 128 * tb : 128 * (tb + 1), :]
                    )
                    a_tiles.append(a_t)
                    if mm1_bf:
                        a_bf = abf_pool.tile([128, D], bf16, tag="abf")
                        nc.vector.tensor_copy(a_bf[:], a_t[:])
                        src_tiles.append(a_bf)
                    else:
                        src_tiles.append(a_t)
            mm2_tiles = src_tiles if mm2_bf else a_tiles

            # mm1: hT[10, T] = sum_c W1a_c.T @ aT_c
            # aT tiles come from PE matmul-transposes (even chunks) and
            # DMA xbar transposes (odd chunks, bf16 SBUF->SBUF) in parallel
            h_ps = ps_h.tile([10, T], f32, tag="h")
            aT_sbs = []
            for c in range(NC1):
                aT_sb = aT_pool.tile([128, T], tdt, tag="aT")
                aT_ps = ps_aT.tile([128, T], f32, tag="aT")
                for tb in range(NTB):
                    # one accumulation group across the 4 disjoint column
                    # slices of this bank — avoids a PSUM drain per block
                    nc.tensor.matmul(
                        aT_ps[:, 128 * tb : 128 * (tb + 1)],
                        lhsT=src_tiles[tb][:, 128 * c : 128 * (c + 1)],
                        rhs=ident_t[:],
                        start=(tb == 0),
                        stop=(tb == NTB - 1),
                        skip_group_check=True,
                    )
                # split the PSUM->SBUF copy across both vector engines to
                # halve its latency on the transpose->mm1 critical path
                nc.vector.tensor_copy(aT_sb[:, 0:256], aT_ps[:, 0:256])
                nc.scalar.copy(aT_sb[:, 256:512], aT_ps[:, 256:512])
                aT_sbs.append(aT_sb)
            # dense mm1 chain after the copies are in flight
            for c in range(NC1):
                nc.tensor.matmul(
                    h_ps[:],
                    lhsT=w1_t[:, 10 * c : 10 * (c + 1)],
                    rhs=aT_sbs[c][:],
                    start=(c == 0),
                    stop=(c == NC1 - 1),
                )

            # tanh(h + s_contrib[:, b] + b1)
            hT_sb = row_pool.tile([10, T], bf16 if mm2_bf else f32, tag="hT")
            nc.scalar.activation(
                hT_sb[:], h_ps[:], AF.Tanh, bias=bias_sb[:, b : b + 1]
            )

            # e = relu(W2.T @ hT + b2); exp + sum; normalize
            e_ps = ps_e.tile([1, T], f32, tag="e")
            nc.tensor.matmul(
                e_ps[:], lhsT=w2_t[:], rhs=hT_sb[:], start=True, stop=True
            )
            er_sb = row_pool.tile([1, T], f32, tag="er")
            nc.scalar.activation(er_sb[:], e_ps[:], AF.Relu, bias=b2_sb[:])
            ex_sb = row_pool.tile([1, T], f32, tag="ex")
            sum_sb = row_pool.tile([1, 1], f32, tag="sum")
            nc.scalar.activation(ex_sb[:], er_sb[:], AF.Exp, accum_out=sum_sb[:])
            rec_sb = row_pool.tile([1, 1], f32, tag="rec")
            nc.vector.reciprocal(rec_sb[:], sum_sb[:])
            if mm2_bf:
                sco_t = row_pool.tile([1, T], bf16, tag="sco_bf")
                nc.vector.tensor_scalar_mul(sco_t[:], ex_sb[:], rec_sb[:])
            sco_sb = row_pool.tile([1, T], f32, tag="sco")
            nc.vector.tensor_scalar_mul(sco_sb[:], ex_sb[:], rec_sb[:])
            nc.sync.dma_start(out=sco_d[b : b + 1, :], in_=sco_sb[:])
            if not mm2_bf:
                sco_t = sco_sb

            # transpose scores -> [128, NTB] via K=1 matmuls
            scT_ps = ps_sT.tile([128, NTB], f32, tag="sT")
            for tb in range(NTB):
                nc.tensor.matmul(
                    scT_ps[:, tb : tb + 1],
                    lhsT=sco_t[:, 128 * tb : 128 * (tb + 1)],
                    rhs=ones11[:],
                    start=(tb == 0),
                    stop=(tb == NTB - 1),
                    skip_group_check=True,
                )
            scT_sb = row_pool.tile([128, NTB], sdt, tag="scT")
            nc.vector.tensor_copy(scT_sb[:], scT_ps[:])

            # mm2: ctx[1, D] = sum_tb scT_tb.T @ a_tb
            # tb-outer so each score-column weight load serves both halves
            ctx_ps = ps_ctx.tile([1, D], f32, tag="ctx")
            for tb in range(NTB):
                for hd in range(2):
                    nc.tensor.matmul(
                        ctx_ps[:, 512 * hd : 512 * (hd + 1)],
                        lhsT=scT_sb[:, tb : tb + 1],
                        rhs=mm2_tiles[tb][:, 512 * hd : 512 * (hd + 1)],
                        start=(tb == 0),
                        stop=(tb == NTB - 1),
                        skip_group_check=True,
                    )
            ctx_sb = row_pool.tile([1, D], f32, tag="ctxr")
            nc.vector.tensor_copy(ctx_sb[:, 0:512], ctx_ps[:, 0:512])
            nc.scalar.copy(ctx_sb[:, 512:1024], ctx_ps[:, 512:1024])
            nc.sync.dma_start(out=ctx_d[b : b + 1, :], in_=ctx_sb[:])

    nc.compile()
    return nc


def get_nc(mode=MODE):
    if mode not in _CACHE:
        _CACHE[mode] = _build(mode)
    return _CACHE[mode]


def make_in_maps(a, s_prev, W1, b1, W2, b2):
    a = np.ascontiguousarray(np.asarray(a, dtype=np.float32))
    s_prev = np.ascontiguousarray(np.asarray(s_prev, dtype=np.float32))
    W1 = np.ascontiguousarray(np.asarray(W1, dtype=np.float32))
    b1 = np.ascontiguousarray(np.asarray(b1, dtype=np.float32))
    W2 = np.ascontiguousarray(np.asarray(W2, dtype=np.float32))
    b2 = np.ascontiguousarray(np.asarray(b2, dtype=np.float32))
    in_maps = []
    for i in range(NCORES):
        sl = slice(i * BC, (i + 1) * BC)
        in_maps.append(
            {
                "a": a[sl],
                "s_prev": s_prev[sl],
                "W1": W1,
                "b1": b1,
                "W2": W2,
                "b2": b2,
            }
        )
    return in_maps


def assemble(results):
    ctx = np.concatenate([r["ctx_out"] for r in results], axis=0)  # [B, D]
    sco = np.concatenate([r["score_out"] for r in results], axis=0)  # [B, T]
    context_vector = ctx.reshape(B, 1, D).astype(np.float32)
    attention_score = sco.reshape(B, T, 1).astype(np.float32)
    return context_vector, attention_score


def run_spmd(inputs, trace=False, mode=MODE, **kwargs):
    from concourse.bass_utils import run_bass_kernel_spmd

    nc = get_nc(mode)
    in_maps = make_in_maps(**inputs)
    res = run_bass_kernel_spmd(nc, in_maps, list(range(NCORES)), trace=trace, **kwargs)
    return res


def kernel(a, s_prev, W1, b1, W2, b2):
    res = run_spmd(dict(a=a, s_prev=s_prev, W1=W1, b1=b1, W2=W2, b2=b2))
    return assemble(res.results)


# revision 31
# speedup vs baseline: 1.0781x; 1.0781x over previous
"""Trainium2 Bass kernel for nn_Attention_76854144795156.

Computes, per batch b:
    h   = tanh(a[b] @ W1a + s_prev[b] @ W1s + b1)      # [T, 10]
    e   = relu(h @ W2 + b2)                            # [T, 1]
    sco = softmax(e, axis=0)                           # [T, 1]
    ctx = sco.T @ a[b]                                 # [1, D]
Returns (context_vector [B,1,D], attention_score [B,T,1]).

Sharding: pure data parallel over batch across 8 NeuronCores
(8 batches per core); the tiny Dense weights are replicated.

The first matmul needs a^T (contraction over the feature dim), which on
the PE requires transposing `a`, and fp32 matmuls/weight-loads run at
half rate (LOW_HIGH 2-pass emulation) with non-overlappable weight
loads.  The kernel therefore streams `a` in bf16 (cast inline by the
DMA on the HBM->SBUF load; accumulation stays fp32 in PSUM), while the
softmax chain and the s_prev contribution are computed in fp32.
"""

import os
import numpy as np
from contextlib import ExitStack

B, T, D, S = 64, 512, 1024, 1024
NCORES = 8
BC = B // NCORES  # batches per core
NTB = T // 128    # t-blocks per batch (4)
NC1 = D // 128    # d-chunks for W1a (8)
NC2 = S // 128    # s-chunks for W1s (8)

MODE = os.environ.get("ATT_KERNEL_MODE", "bf16_all")  # fp32 | bf16_mm1 | bf16_all

_CACHE = {}


def _build(mode):
    import concourse.tile as tile
    from concourse import bacc, mybir
    from concourse.masks import make_identity

    f32 = mybir.dt.float32
    bf16 = mybir.dt.bfloat16
    AF = mybir.ActivationFunctionType
    AX = mybir.AxisListType
    ALU = mybir.AluOpType
    mm1_bf = mode != "fp32"
    mm2_bf = mode == "bf16_all"

    nc = bacc.Bacc("TRN2", num_devices=NCORES)

    a_d = nc.dram_tensor("a", [BC, T, D], f32, kind="ExternalInput")
    s_d = nc.dram_tensor("s_prev", [BC, S], f32, kind="ExternalInput")
    w1_d = nc.dram_tensor("W1", [D + S, 10], f32, kind="ExternalInput")
    b1_d = nc.dram_tensor("b1", [10], f32, kind="ExternalInput")
    w2_d = nc.dram_tensor("W2", [10, 1], f32, kind="ExternalInput")
    b2_d = nc.dram_tensor("b2", [1], f32, kind="ExternalInput")
    ctx_d = nc.dram_tensor("ctx_out", [BC, D], f32, kind="ExternalOutput")
    sco_d = nc.dram_tensor("score_out", [BC, T], f32, kind="ExternalOutput")

    with tile.TileContext(nc) as tc, ExitStack() as ctx:
        const_pool = ctx.enter_context(tc.tile_pool(name="const", bufs=1))
        a_pool = ctx.enter_context(
            tc.tile_pool(name="a_res", bufs=(8 if mm2_bf else 4 * BC))
        )
        abf_pool = ctx.enter_context(
            tc.tile_pool(name="a_bf", bufs=(4 * BC if mm2_bf else 8))
        )
        aT_pool = ctx.enter_context(tc.tile_pool(name="aT", bufs=10))
        row_pool = ctx.enter_context(tc.tile_pool(name="rows", bufs=3))
        ps_aT = ctx.enter_context(tc.tile_pool(name="ps_aT", bufs=2, space="PSUM"))
        ps_h = ctx.enter_context(tc.tile_pool(name="ps_h", bufs=2, space="PSUM"))
        ps_e = ctx.enter_context(tc.tile_pool(name="ps_e", bufs=1, space="PSUM"))
        ps_sT = ctx.enter_context(tc.tile_pool(name="ps_sT", bufs=1, space="PSUM"))
        ps_ctx = ctx.enter_context(tc.tile_pool(name="ps_ctx", bufs=1, space="PSUM"))

        # ---- issue the big streaming loads first: SWDGE starts filling
        # SBUF while the identity/constant preamble runs on other engines ----
        tdt = bf16 if mm1_bf else f32
        sdt = bf16 if mm2_bf else f32
        # ---- identity: bulk zero on DVE, diagonal fill on GpSimd (the
        # only gpsimd preamble op, so the SWDGE loads issue right after) ----
        def _make_identity(ap):
            nc.vector.memset(ap, 0.0)
            nc.gpsimd.affine_select(
                out=ap,
                in_=ap,
                compare_op=mybir.AluOpType.not_equal,
                fill=1.0,
                base=0,
                pattern=[[-1, ap.shape[0]]],
                channel_multiplier=1,
            )

        ident8 = const_pool.tile([BC, BC], sdt, tag="ident8")
        _make_identity(ident8[:])
        if mm1_bf:
            ident_t = const_pool.tile([128, 128], bf16, tag="ident_bf")
            _make_identity(ident_t[:])
            ident = ident_t if mm2_bf else const_pool.tile([128, 128], f32, tag="ident")
            if not mm2_bf:
                _make_identity(ident[:])
        else:
            ident = const_pool.tile([128, 128], f32, tag="ident")
            _make_identity(ident[:])
            ident_t = ident

        # ---- big streaming loads (SWDGE casts fp32->bf16 inline) ----
        all_src = {}
        s_sb = const_pool.tile([BC, S], sdt, tag="s")
        if mm2_bf:
            nc.gpsimd.dma_start(out=s_sb[:], in_=s_d[:, :])
            for b in range(BC):
                for tb in range(NTB):
                    a_bf = abf_pool.tile([128, D], bf16, tag="abf", name=f"abf{b}_{tb}")
                    nc.gpsimd.dma_start(
                        out=a_bf[:], in_=a_d[b, 128 * tb : 128 * (tb + 1), :]
                    )
                    all_src[(b, tb)] = a_bf
        else:
            nc.sync.dma_start(out=s_sb[:], in_=s_d[:, :])
        ones11 = const_pool.tile([1, 1], sdt, tag="ones11")
        nc.vector.memset(ones11[:], 1.0)

        w1_sb = const_pool.tile([128, 160], f32, tag="w1")  # 16 chunks of 10 cols
        nc.sync.dma_start(
            out=w1_sb.rearrange("p (c u) -> p c u", u=10),
            in_=w1_d.rearrange("(c p) u -> p c u", p=128),
        )
        if mm1_bf:
            w1_t = const_pool.tile([128, 160], bf16, tag="w1_bf")
            nc.vector.tensor_copy(w1_t[:], w1_sb[:])
        else:
            w1_t = w1_sb
        w2_sb = const_pool.tile([10, 1], f32, tag="w2")
        nc.sync.dma_start(out=w2_sb[:], in_=w2_d[:, :])
        if mm2_bf:
            w2_t = const_pool.tile([10, 1], bf16, tag="w2_bf")
            nc.vector.tensor_copy(w2_t[:], w2_sb[:])
        else:
            w2_t = w2_sb
        b1_sb = const_pool.tile([10, 1], f32, tag="b1")
        nc.sync.dma_start(out=b1_sb[:], in_=b1_d.rearrange("(u o) -> u o", o=1))
        b2_sb = const_pool.tile([1, 1], f32, tag="b2")
        nc.sync.dma_start(out=b2_sb[:], in_=b2_d.rearrange("(u o) -> u o", o=1))

        # ---- s_contrib = W1s.T @ s_prev.T -> [10, BC] ----
        sT_sb = const_pool.tile([128, NC2 * BC], sdt, tag="sT")
        for c in range(NC2):
            sT_ps = ps_sT.tile([128, BC], f32, tag="sT")
            nc.tensor.matmul(
                sT_ps[:],
                lhsT=s_sb[:, 128 * c : 128 * (c + 1)],
                rhs=ident8[:],
                start=True,
                stop=True,
            )
            nc.vector.tensor_copy(sT_sb[:, BC * c : BC * (c + 1)], sT_ps[:])
        sc_ps = ps_h.tile([10, BC], f32, tag="h")
        for c in range(NC2):
            nc.tensor.matmul(
                sc_ps[:],
                lhsT=(w1_t if mm2_bf else w1_sb)[
                    :, 10 * (NC1 + c) : 10 * (NC1 + c) + 10
                ],
                rhs=sT_sb[:, BC * c : BC * (c + 1)],
                start=(c == 0),
                stop=(c == NC2 - 1),
            )
        bias_sb = const_pool.tile([10, BC], f32, tag="bias")
        nc.scalar.activation(bias_sb[:], sc_ps[:], AF.Identity, bias=b1_sb[:])

        # ---- per-batch main loop ----
        for b in range(BC):
            if mm2_bf:
                # loaded (with inline fp32->bf16 cast) up front
                src_tiles = [all_src[(b, tb)] for tb in range(NTB)]
            else:
                src_tiles = []
                a_tiles = []
                for tb in range(NTB):
                    a_t = a_pool.tile([128, D], f32, tag="a")
                    nc.sync.dma_start(
                        out=a_t[:], in_=a_d[b, 128 * tb : 128 * (tb + 1), :]
                    )
                    a_tiles.append(a_t)
                    if mm1_bf:
                        a_bf = abf_pool.tile([128, D], bf16, tag="abf")
                        nc.vector.tensor_copy(a_bf[:], a_t[:])
                        src_tiles.append(a_bf)
                    else:
                        src_tiles.append(a_t)
            mm2_tiles = src_tiles if mm2_bf else a_tiles

            # mm1: hT[10, T] = sum_c W1a_c.T @ aT_c
            # aT tiles come from PE matmul-transposes (even chunks) and
            # DMA xbar transposes (odd chunks, bf16 SBUF->SBUF) in parallel
            h_ps = ps_h.tile([10, T], f32, tag="h")
            aT_sbs = []
            for c in range(NC1):
                aT_sb = aT_pool.tile([128, T], tdt, tag="aT")
                aT_ps = ps_aT.tile([128, T], f32, tag="aT")
                for tb in range(NTB):
                    # one accumulation group across the 4 disjoint column
                    # slices of this bank — avoids a PSUM drain per block
                    nc.tensor.matmul(
                        aT_ps[:, 128 * tb : 128 * (tb + 1)],
                        lhsT=src_tiles[tb][:, 128 * c : 128 * (c + 1)],
                        rhs=ident_t[:],
                        start=(tb == 0),
                        stop=(tb == NTB - 1),
                        skip_group_check=True,
                    )
                if c % 2 == 0:
                    nc.vector.tensor_copy(aT_sb[:], aT_ps[:])
                else:
                    nc.scalar.copy(aT_sb[:], aT_ps[:])
                aT_sbs.append(aT_sb)
            # dense mm1 chain after the copies are in flight
            for c in range(NC1):
                nc.tensor.matmul(
                    h_ps[:],
                    lhsT=w1_t[:, 10 * c : 10 * (c + 1)],
                    rhs=aT_sbs[c][:],
                    start=(c == 0),
                    stop=(c == NC1 - 1),
                )

            # tanh(h + s_contrib[:, b] + b1)
            hT_sb = row_pool.tile([10, T], bf16 if mm2_bf else f32, tag="hT")
            nc.scalar.activation(
                hT_sb[:], h_ps[:], AF.Tanh, bias=bias_sb[:, b : b + 1]
            )

            # e = relu(W2.T @ hT + b2); exp + sum; normalize
            e_ps = ps_e.tile([1, T], f32, tag="e")
            nc.tensor.matmul(
                e_ps[:], lhsT=w2_t[:], rhs=hT_sb[:], start=True, stop=True
            )
            er_sb = row_pool.tile([1, T], f32, tag="er")
            nc.scalar.activation(er_sb[:], e_ps[:], AF.Relu, bias=b2_sb[:])
            ex_sb = row_pool.tile([1, T], f32, tag="ex")
            sum_sb = row_pool.tile([1, 1], f32, tag="sum")
            nc.scalar.activation(ex_sb[:], er_sb[:], AF.Exp, accum_out=sum_sb[:])
            rec_sb = row_pool.tile([1, 1], f32, tag="rec")
            nc.vector.reciprocal(rec_sb[:], sum_sb[:])
            if mm2_bf:
                sco_t = row_pool.tile([1, T], bf16, tag="sco_bf")
                nc.vector.tensor_scalar_mul(sco_t[:], ex_sb[:], rec_sb[:])
            sco_sb = row_pool.tile([1, T], f32, tag="sco")
            nc.vector.tensor_scalar_mul(sco_sb[:], ex_sb[:], rec_sb[:])
            nc.sync.dma_start(out=sco_d[b : b + 1, :], in_=sco_sb[:])
            if not mm2_bf:
                sco_t = sco_sb

            # transpose scores -> [128, NTB] via K=1 matmuls
            scT_ps = ps_sT.tile([128, NTB], f32, tag="sT")
            for tb in range(NTB):
                nc.tensor.matmul(
                    scT_ps[:, tb : tb + 1],
                    lhsT=sco_t[:, 128 * tb : 128 * (tb + 1)],
                    rhs=ones11[:],
                    start=(tb == 0),
                    stop=(tb == NTB - 1),
                    skip_group_check=True,
                )
            scT_sb = row_pool.tile([128, NTB], sdt, tag="scT")
            nc.vector.tensor_copy(scT_sb[:], scT_ps[:])

            # mm2: ctx[1, D] = sum_tb scT_tb.T @ a_tb
            # tb-outer so each score-column weight load serves both halves
            ctx_ps = ps_ctx.tile([1, D], f32, tag="ctx")
            for tb in range(NTB):
                for hd in range(2):
                    nc.tensor.matmul(
                        ctx_ps[:, 512 * hd : 512 * (hd + 1)],
                        lhsT=scT_sb[:, tb : tb + 1],
                        rhs=mm2_tiles[tb][:, 512 * hd : 512 * (hd + 1)],
                        start=(tb == 0),
                        stop=(tb == NTB - 1),
                        skip_group_check=True,
                    )
            ctx_sb = row_pool.tile([1, D], f32, tag="ctxr")
            nc.vector.tensor_copy(ctx_sb[:, 0:512], ctx_ps[:, 0:512])
            nc.scalar.copy(ctx_sb[:, 512:1024], ctx_ps[:, 512:1024])
            nc.sync.dma_start(out=ctx_d[b : b + 1, :], in_=ctx_sb[:])

    nc.compile()
    return nc


def get_nc(mode=MODE):
    if mode not in _CACHE:
        _CACHE[mode] = _build(mode)
    return _CACHE[mode]


def make_in_maps(a, s_prev, W1, b1, W2, b2):
    a = np.ascontiguousarray(np.asarray(a, dtype=np.float32))
    s_prev = np.ascontiguousarray(np.asarray(s_prev, dtype=np.float32))
    W1 = np.ascontiguousarray(np.asarray(W1, dtype=np.float32))
    b1 = np.ascontiguousarray(np.asarray(b1, dtype=np.float32))
    W2 = np.ascontiguousarray(np.asarray(W2, dtype=np.float32))
    b2 = np.ascontiguousarray(np.asarray(b2, dtype=np.float32))
    in_maps = []
    for i in range(NCORES):
        sl = slice(i * BC, (i + 1) * BC)
        in_maps.append(
            {
                "a": a[sl],
                "s_prev": s_prev[sl],
                "W1": W1,
                "b1": b1,
                "W2": W2,
                "b2": b2,
            }
        )
    return in_maps


def assemble(results):
    ctx = np.concatenate([r["ctx_out"] for r in results], axis=0)  # [B, D]
    sco = np.concatenate([r["score_out"] for r in results], axis=0)  # [B, T]
    context_vector = ctx.reshape(B, 1, D).astype(np.float32)
    attention_score = sco.reshape(B, T, 1).astype(np.float32)
    return context_vector, attention_score


def run_spmd(inputs, trace=False, mode=MODE, **kwargs):
    from concourse.bass_utils import run_bass_kernel_spmd

    nc = get_nc(mode)
    in_maps = make_in_maps(**inputs)
    res = run_bass_kernel_spmd(nc, in_maps, list(range(NCORES)), trace=trace, **kwargs)
    return res


def kernel(a, s_prev, W1, b1, W2, b2):
    res = run_spmd(dict(a=a, s_prev=s_prev, W1=W1, b1=b1, W2=W2, b2=b2))
    return assemble(res.results)


# revision 33
# speedup vs baseline: 1.1261x; 1.0446x over previous
"""Trainium2 Bass kernel for nn_Attention_76854144795156.

Computes, per batch b:
    h   = tanh(a[b] @ W1a + s_prev[b] @ W1s + b1)      # [T, 10]
    e   = relu(h @ W2 + b2)                            # [T, 1]
    sco = softmax(e, axis=0)                           # [T, 1]
    ctx = sco.T @ a[b]                                 # [1, D]
Returns (context_vector [B,1,D], attention_score [B,T,1]).

Sharding: pure data parallel over batch across 8 NeuronCores
(8 batches per core); the tiny Dense weights are replicated.

The first matmul needs a^T (contraction over the feature dim), which on
the PE requires transposing `a`, and fp32 matmuls/weight-loads run at
half rate (LOW_HIGH 2-pass emulation) with non-overlappable weight
loads.  The kernel therefore streams `a` in bf16 (cast inline by the
DMA on the HBM->SBUF load; accumulation stays fp32 in PSUM), while the
softmax chain and the s_prev contribution are computed in fp32.
"""

import os
import numpy as np
from contextlib import ExitStack

B, T, D, S = 64, 512, 1024, 1024
NCORES = 8
BC = B // NCORES  # batches per core
NTB = T // 128    # t-blocks per batch (4)
NC1 = D // 128    # d-chunks for W1a (8)
NC2 = S // 128    # s-chunks for W1s (8)

MODE = os.environ.get("ATT_KERNEL_MODE", "bf16_all")  # fp32 | bf16_mm1 | bf16_all

_CACHE = {}


def _build(mode):
    import concourse.tile as tile
    from concourse import bacc, mybir
    from concourse.masks import make_identity

    f32 = mybir.dt.float32
    bf16 = mybir.dt.bfloat16
    AF = mybir.ActivationFunctionType
    AX = mybir.AxisListType
    ALU = mybir.AluOpType
    mm1_bf = mode != "fp32"
    mm2_bf = mode == "bf16_all"

    nc = bacc.Bacc("TRN2", num_devices=NCORES)

    a_d = nc.dram_tensor("a", [BC, T, D], f32, kind="ExternalInput")
    s_d = nc.dram_tensor("s_prev", [BC, S], f32, kind="ExternalInput")
    w1_d = nc.dram_tensor("W1", [D + S, 10], f32, kind="ExternalInput")
    b1_d = nc.dram_tensor("b1", [10], f32, kind="ExternalInput")
    w2_d = nc.dram_tensor("W2", [10, 1], f32, kind="ExternalInput")
    b2_d = nc.dram_tensor("b2", [1], f32, kind="ExternalInput")
    ctx_d = nc.dram_tensor("ctx_out", [BC, D], f32, kind="ExternalOutput")
    sco_d = nc.dram_tensor("score_out", [BC, T], f32, kind="ExternalOutput")

    with tile.TileContext(nc) as tc, ExitStack() as ctx:
        const_pool = ctx.enter_context(tc.tile_pool(name="const", bufs=1))
        a_pool = ctx.enter_context(
            tc.tile_pool(name="a_res", bufs=(8 if mm2_bf else 4 * BC))
        )
        abf_pool = ctx.enter_context(
            tc.tile_pool(name="a_bf", bufs=(4 * BC if mm2_bf else 8))
        )
        aT_pool = ctx.enter_context(tc.tile_pool(name="aT", bufs=10))
        row_pool = ctx.enter_context(tc.tile_pool(name="rows", bufs=3))
        ps_aT = ctx.enter_context(tc.tile_pool(name="ps_aT", bufs=2, space="PSUM"))
        ps_h = ctx.enter_context(tc.tile_pool(name="ps_h", bufs=2, space="PSUM"))
        ps_e = ctx.enter_context(tc.tile_pool(name="ps_e", bufs=1, space="PSUM"))
        ps_sT = ctx.enter_context(tc.tile_pool(name="ps_sT", bufs=1, space="PSUM"))
        ps_ctx = ctx.enter_context(tc.tile_pool(name="ps_ctx", bufs=1, space="PSUM"))

        # ---- issue the big streaming loads first: SWDGE starts filling
        # SBUF while the identity/constant preamble runs on other engines ----
        tdt = bf16 if mm1_bf else f32
        sdt = bf16 if mm2_bf else f32
        # ---- identity: bulk zero on DVE, diagonal fill on GpSimd (the
        # only gpsimd preamble op, so the SWDGE loads issue right after) ----
        def _make_identity(ap):
            nc.vector.memset(ap, 0.0)
            nc.gpsimd.affine_select(
                out=ap,
                in_=ap,
                compare_op=mybir.AluOpType.not_equal,
                fill=1.0,
                base=0,
                pattern=[[-1, ap.shape[0]]],
                channel_multiplier=1,
            )

        ident8 = const_pool.tile([BC, BC], sdt, tag="ident8")
        _make_identity(ident8[:])
        if mm1_bf:
            ident_t = const_pool.tile([128, 128], bf16, tag="ident_bf")
            _make_identity(ident_t[:])
            ident = ident_t if mm2_bf else const_pool.tile([128, 128], f32, tag="ident")
            if not mm2_bf:
                _make_identity(ident[:])
        else:
            ident = const_pool.tile([128, 128], f32, tag="ident")
            _make_identity(ident[:])
            ident_t = ident

        # ---- big streaming loads (SWDGE casts fp32->bf16 inline) ----
        all_src = {}
        s_sb = const_pool.tile([BC, S], sdt, tag="s")
        if mm2_bf:
            nc.gpsimd.dma_start(out=s_sb[:], in_=s_d[:, :])
            for b in range(BC):
                for tb in range(NTB):
                    a_bf = abf_pool.tile([128, D], bf16, tag="abf", name=f"abf{b}_{tb}")
                    nc.gpsimd.dma_start(
                        out=a_bf[:], in_=a_d[b, 128 * tb : 128 * (tb + 1), :]
                    )
                    all_src[(b, tb)] = a_bf
        else:
            nc.sync.dma_start(out=s_sb[:], in_=s_d[:, :])
        ones11 = const_pool.tile([1, 1], sdt, tag="ones11")
        nc.vector.memset(ones11[:], 1.0)

        w1_sb = const_pool.tile([128, 160], f32, tag="w1")  # 16 chunks of 10 cols
        nc.sync.dma_start(
            out=w1_sb.rearrange("p (c u) -> p c u", u=10),
            in_=w1_d.rearrange("(c p) u -> p c u", p=128),
        )
        if mm1_bf:
            w1_t = const_pool.tile([128, 160], bf16, tag="w1_bf")
            nc.vector.tensor_copy(w1_t[:], w1_sb[:])
        else:
            w1_t = w1_sb
        w2_sb = const_pool.tile([10, 1], f32, tag="w2")
        nc.sync.dma_start(out=w2_sb[:], in_=w2_d[:, :])
        if mm2_bf:
            w2_t = const_pool.tile([10, 1], bf16, tag="w2_bf")
            nc.vector.tensor_copy(w2_t[:], w2_sb[:])
        else:
            w2_t = w2_sb
        b1_sb = const_pool.tile([10, 1], f32, tag="b1")
        nc.sync.dma_start(out=b1_sb[:], in_=b1_d.rearrange("(u o) -> u o", o=1))
        b2_sb = const_pool.tile([1, 1], f32, tag="b2")
        nc.sync.dma_start(out=b2_sb[:], in_=b2_d.rearrange("(u o) -> u o", o=1))

        # ---- s_contrib = W1s.T @ s_prev.T -> [10, BC] ----
        sT_sb = const_pool.tile([128, NC2 * BC], sdt, tag="sT")
        for c in range(NC2):
            sT_ps = ps_sT.tile([128, BC], f32, tag="sT")
            nc.tensor.matmul(
                sT_ps[:],
                lhsT=s_sb[:, 128 * c : 128 * (c + 1)],
                rhs=ident8[:],
                start=True,
                stop=True,
            )
            nc.vector.tensor_copy(sT_sb[:, BC * c : BC * (c + 1)], sT_ps[:])
        sc_ps = ps_h.tile([10, BC], f32, tag="h")
        for c in range(NC2):
            nc.tensor.matmul(
                sc_ps[:],
                lhsT=(w1_t if mm2_bf else w1_sb)[
                    :, 10 * (NC1 + c) : 10 * (NC1 + c) + 10
                ],
                rhs=sT_sb[:, BC * c : BC * (c + 1)],
                start=(c == 0),
                stop=(c == NC2 - 1),
            )
        bias_sb = const_pool.tile([10, BC], f32, tag="bias")
        nc.scalar.activation(bias_sb[:], sc_ps[:], AF.Identity, bias=b1_sb[:])

        # ---- per-batch main loop ----
        for b in range(BC):
            if mm2_bf:
                # loaded (with inline fp32->bf16 cast) up front
                src_tiles = [all_src[(b, tb)] for tb in range(NTB)]
            else:
                src_tiles = []
                a_tiles = []
                for tb in range(NTB):
                    a_t = a_pool.tile([128, D], f32, tag="a")
                    nc.sync.dma_start(
                        out=a_t[:], in_=a_d[b, 128 * tb : 128 * (tb + 1), :]
                    )
                    a_tiles.append(a_t)
                    if mm1_bf:
                        a_bf = abf_pool.tile([128, D], bf16, tag="abf")
                        nc.vector.tensor_copy(a_bf[:], a_t[:])
                        src_tiles.append(a_bf)
                    else:
                        src_tiles.append(a_t)
            mm2_tiles = src_tiles if mm2_bf else a_tiles

            # mm1: hT[10, T] = sum_c W1a_c.T @ aT_c
            # aT tiles come from PE matmul-transposes (even chunks) and
            # DMA xbar transposes (odd chunks, bf16 SBUF->SBUF) in parallel
            h_ps = ps_h.tile([10, T], f32, tag="h")
            aT_sbs = []
            for c in range(NC1):
                aT_sb = aT_pool.tile([128, T], tdt, tag="aT")
                aT_ps = ps_aT.tile([128, T], f32, tag="aT")
                for tb in range(NTB):
                    # one accumulation group across the 4 disjoint column
                    # slices of this bank — avoids a PSUM drain per block
                    nc.tensor.matmul(
                        aT_ps[:, 128 * tb : 128 * (tb + 1)],
                        lhsT=src_tiles[tb][:, 128 * c : 128 * (c + 1)],
                        rhs=ident_t[:],
                        start=(tb == 0),
                        stop=(tb == NTB - 1),
                        skip_group_check=True,
                    )
                if c % 2 == 0:
                    nc.vector.tensor_copy(aT_sb[:], aT_ps[:])
                else:
                    nc.scalar.copy(aT_sb[:], aT_ps[:])
                aT_sbs.append(aT_sb)
            # dense mm1 chain after the copies are in flight
            for c in range(NC1):
                nc.tensor.matmul(
                    h_ps[:],
                    lhsT=w1_t[:, 10 * c : 10 * (c + 1)],
                    rhs=aT_sbs[c][:],
                    start=(c == 0),
                    stop=(c == NC1 - 1),
                )

            # tanh(h + s_contrib[:, b] + b1)
            hT_sb = row_pool.tile([10, T], bf16 if mm2_bf else f32, tag="hT")
            nc.scalar.activation(
                hT_sb[:], h_ps[:], AF.Tanh, bias=bias_sb[:, b : b + 1]
            )

            # e = relu(W2.T @ hT + b2); exp + sum; normalize
            e_ps = ps_e.tile([1, T], f32, tag="e")
            nc.tensor.matmul(
                e_ps[:], lhsT=w2_t[:], rhs=hT_sb[:], start=True, stop=True
            )
            # exp(relu(x + b2)) == max(exp(x + b2), 1); fold the relu into a
            # DVE max that also produces the softmax denominator
            ee_sb = row_pool.tile([1, T], f32, tag="ee")
            nc.scalar.activation(ee_sb[:], e_ps[:], AF.Exp, bias=b2_sb[:])
            ex_sb = row_pool.tile([1, T], f32, tag="ex")
            sum_sb = row_pool.tile([1, 1], f32, tag="sum")
            nc.vector.tensor_scalar(
                ex_sb[:],
                ee_sb[:],
                1.0,
                None,
                op0=mybir.AluOpType.max,
                op1=mybir.AluOpType.add,
                accum_out=sum_sb[:],
            )
            rec_sb = row_pool.tile([1, 1], f32, tag="rec")
            nc.vector.reciprocal(rec_sb[:], sum_sb[:])
            if mm2_bf:
                sco_t = row_pool.tile([1, T], bf16, tag="sco_bf")
                nc.vector.tensor_scalar_mul(sco_t[:], ex_sb[:], rec_sb[:])
            sco_sb = row_pool.tile([1, T], f32, tag="sco")
            nc.vector.tensor_scalar_mul(sco_sb[:], ex_sb[:], rec_sb[:])
            nc.sync.dma_start(out=sco_d[b : b + 1, :], in_=sco_sb[:])
            if not mm2_bf:
                sco_t = sco_sb

            # transpose scores -> [128, NTB] via K=1 matmuls
            scT_ps = ps_sT.tile([128, NTB], f32, tag="sT")
            for tb in range(NTB):
                nc.tensor.matmul(
                    scT_ps[:, tb : tb + 1],
                    lhsT=sco_t[:, 128 * tb : 128 * (tb + 1)],
                    rhs=ones11[:],
                    start=(tb == 0),
                    stop=(tb == NTB - 1),
                    skip_group_check=True,
                )
            scT_sb = row_pool.tile([128, NTB], sdt, tag="scT")
            nc.vector.tensor_copy(scT_sb[:], scT_ps[:])

            # mm2: ctx[1, D] = sum_tb scT_tb.T @ a_tb
            # tb-outer so each score-column weight load serves both halves
            ctx_ps = ps_ctx.tile([1, D], f32, tag="ctx")
            for tb in range(NTB):
                for hd in range(2):
                    nc.tensor.matmul(
                        ctx_ps[:, 512 * hd : 512 * (hd + 1)],
                        lhsT=scT_sb[:, tb : tb + 1],
                        rhs=mm2_tiles[tb][:, 512 * hd : 512 * (hd + 1)],
                        start=(tb == 0),
                        stop=(tb == NTB - 1),
                        skip_group_check=True,
                    )
            ctx_sb = row_pool.tile([1, D], f32, tag="ctxr")
            nc.vector.tensor_copy(ctx_sb[:, 0:512], ctx_ps[:, 0:512])
            nc.scalar.copy(ctx_sb[:, 512:1024], ctx_ps[:, 512:1024])
            nc.sync.dma_start(out=ctx_d[b : b + 1, :], in_=ctx_sb[:])

    nc.compile()
    return nc


def get_nc(mode=MODE):
    if mode not in _CACHE:
        _CACHE[mode] = _build(mode)
    return _CACHE[mode]


def make_in_maps(a, s_prev, W1, b1, W2, b2):
    a = np.ascontiguousarray(np.asarray(a, dtype=np.float32))
    s_prev = np.ascontiguousarray(np.asarray(s_prev, dtype=np.float32))
    W1 = np.ascontiguousarray(np.asarray(W1, dtype=np.float32))
    b1 = np.ascontiguousarray(np.asarray(b1, dtype=np.float32))
    W2 = np.ascontiguousarray(np.asarray(W2, dtype=np.float32))
    b2 = np.ascontiguousarray(np.asarray(b2, dtype=np.float32))
    in_maps = []
    for i in range(NCORES):
        sl = slice(i * BC, (i + 1) * BC)
        in_maps.append(
            {
                "a": a[sl],
                "s_prev": s_prev[sl],
                "W1": W1,
                "b1": b1,
                "W2": W2,
                "b2": b2,
            }
        )
    return in_maps


def assemble(results):
    ctx = np.concatenate([r["ctx_out"] for r in results], axis=0)  # [B, D]
    sco = np.concatenate([r["score_out"] for r in results], axis=0)  # [B, T]
    context_vector = ctx.reshape(B, 1, D).astype(np.float32)
    attention_score = sco.reshape(B, T, 1).astype(np.float32)
    return context_vector, attention_score


def run_spmd(inputs, trace=False, mode=MODE, **kwargs):
    from concourse.bass_utils import run_bass_kernel_spmd

    nc = get_nc(mode)
    in_maps = make_in_maps(**inputs)
    res = run_bass_kernel_spmd(nc, in_maps, list(range(NCORES)), trace=trace, **kwargs)
    return res


def kernel(a, s_prev, W1, b1, W2, b2):
    res = run_spmd(dict(a=a, s_prev=s_prev, W1=W1, b1=b1, W2=W2, b2=b2))
    return assemble(res.results)


# revision 34
# speedup vs baseline: 1.2212x; 1.0845x over previous
"""Trainium2 Bass kernel for nn_Attention_76854144795156.

Computes, per batch b:
    h   = tanh(a[b] @ W1a + s_prev[b] @ W1s + b1)      # [T, 10]
    e   = relu(h @ W2 + b2)                            # [T, 1]
    sco = softmax(e, axis=0)                           # [T, 1]
    ctx = sco.T @ a[b]                                 # [1, D]
Returns (context_vector [B,1,D], attention_score [B,T,1]).

Sharding: pure data parallel over batch across 8 NeuronCores
(8 batches per core); the tiny Dense weights are replicated.

The first matmul needs a^T (contraction over the feature dim), which on
the PE requires transposing `a`, and fp32 matmuls/weight-loads run at
half rate (LOW_HIGH 2-pass emulation) with non-overlappable weight
loads.  The kernel therefore streams `a` in bf16 (cast inline by the
DMA on the HBM->SBUF load; accumulation stays fp32 in PSUM), while the
softmax chain and the s_prev contribution are computed in fp32.
"""

import os
import numpy as np
from contextlib import ExitStack

B, T, D, S = 64, 512, 1024, 1024
NCORES = 8
BC = B // NCORES  # batches per core
NTB = T // 128    # t-blocks per batch (4)
NC1 = D // 128    # d-chunks for W1a (8)
NC2 = S // 128    # s-chunks for W1s (8)

MODE = os.environ.get("ATT_KERNEL_MODE", "bf16_all")  # fp32 | bf16_mm1 | bf16_all

_CACHE = {}


def _build(mode):
    import concourse.tile as tile
    from concourse import bacc, mybir
    from concourse.masks import make_identity

    f32 = mybir.dt.float32
    bf16 = mybir.dt.bfloat16
    AF = mybir.ActivationFunctionType
    AX = mybir.AxisListType
    ALU = mybir.AluOpType
    mm1_bf = mode != "fp32"
    mm2_bf = mode == "bf16_all"

    nc = bacc.Bacc("TRN2", num_devices=NCORES)

    a_d = nc.dram_tensor("a", [BC, T, D], f32, kind="ExternalInput")
    s_d = nc.dram_tensor("s_prev", [BC, S], f32, kind="ExternalInput")
    w1_d = nc.dram_tensor("W1", [D + S, 10], f32, kind="ExternalInput")
    b1_d = nc.dram_tensor("b1", [10], f32, kind="ExternalInput")
    w2_d = nc.dram_tensor("W2", [10, 1], f32, kind="ExternalInput")
    b2_d = nc.dram_tensor("b2", [1], f32, kind="ExternalInput")
    ctx_d = nc.dram_tensor("ctx_out", [BC, D], f32, kind="ExternalOutput")
    sco_d = nc.dram_tensor("score_out", [BC, T], f32, kind="ExternalOutput")

    with tile.TileContext(nc) as tc, ExitStack() as ctx:
        const_pool = ctx.enter_context(tc.tile_pool(name="const", bufs=1))
        a_pool = ctx.enter_context(
            tc.tile_pool(name="a_res", bufs=(8 if mm2_bf else 4 * BC))
        )
        abf_pool = ctx.enter_context(
            tc.tile_pool(name="a_bf", bufs=(4 * BC if mm2_bf else 8))
        )
        aT_pool = ctx.enter_context(tc.tile_pool(name="aT", bufs=10))
        row_pool = ctx.enter_context(tc.tile_pool(name="rows", bufs=3))
        ps_aT = ctx.enter_context(tc.tile_pool(name="ps_aT", bufs=3, space="PSUM"))
        ps_h = ctx.enter_context(tc.tile_pool(name="ps_h", bufs=1, space="PSUM"))
        ps_e = ctx.enter_context(tc.tile_pool(name="ps_e", bufs=1, space="PSUM"))
        ps_sT = ctx.enter_context(tc.tile_pool(name="ps_sT", bufs=1, space="PSUM"))
        ps_ctx = ctx.enter_context(tc.tile_pool(name="ps_ctx", bufs=1, space="PSUM"))

        # ---- issue the big streaming loads first: SWDGE starts filling
        # SBUF while the identity/constant preamble runs on other engines ----
        tdt = bf16 if mm1_bf else f32
        sdt = bf16 if mm2_bf else f32
        # ---- identity: bulk zero on DVE, diagonal fill on GpSimd (the
        # only gpsimd preamble op, so the SWDGE loads issue right after) ----
        def _make_identity(ap):
            nc.vector.memset(ap, 0.0)
            nc.gpsimd.affine_select(
                out=ap,
                in_=ap,
                compare_op=mybir.AluOpType.not_equal,
                fill=1.0,
                base=0,
                pattern=[[-1, ap.shape[0]]],
                channel_multiplier=1,
            )

        ident8 = const_pool.tile([BC, BC], sdt, tag="ident8")
        _make_identity(ident8[:])
        if mm1_bf:
            ident_t = const_pool.tile([128, 128], bf16, tag="ident_bf")
            _make_identity(ident_t[:])
            ident = ident_t if mm2_bf else const_pool.tile([128, 128], f32, tag="ident")
            if not mm2_bf:
                _make_identity(ident[:])
        else:
            ident = const_pool.tile([128, 128], f32, tag="ident")
            _make_identity(ident[:])
            ident_t = ident

        # ---- big streaming loads (SWDGE casts fp32->bf16 inline) ----
        all_src = {}
        s_sb = const_pool.tile([BC, S], sdt, tag="s")
        if mm2_bf:
            nc.gpsimd.dma_start(out=s_sb[:], in_=s_d[:, :])
            for b in range(BC):
                for tb in range(NTB):
                    a_bf = abf_pool.tile([128, D], bf16, tag="abf", name=f"abf{b}_{tb}")
                    nc.gpsimd.dma_start(
                        out=a_bf[:], in_=a_d[b, 128 * tb : 128 * (tb + 1), :]
                    )
                    all_src[(b, tb)] = a_bf
        else:
            nc.sync.dma_start(out=s_sb[:], in_=s_d[:, :])
        ones11 = const_pool.tile([1, 1], sdt, tag="ones11")
        nc.vector.memset(ones11[:], 1.0)

        w1_sb = const_pool.tile([128, 160], f32, tag="w1")  # 16 chunks of 10 cols
        nc.sync.dma_start(
            out=w1_sb.rearrange("p (c u) -> p c u", u=10),
            in_=w1_d.rearrange("(c p) u -> p c u", p=128),
        )
        if mm1_bf:
            w1_t = const_pool.tile([128, 160], bf16, tag="w1_bf")
            nc.vector.tensor_copy(w1_t[:], w1_sb[:])
        else:
            w1_t = w1_sb
        w2_sb = const_pool.tile([10, 1], f32, tag="w2")
        nc.sync.dma_start(out=w2_sb[:], in_=w2_d[:, :])
        if mm2_bf:
            w2_t = const_pool.tile([10, 1], bf16, tag="w2_bf")
            nc.vector.tensor_copy(w2_t[:], w2_sb[:])
        else:
            w2_t = w2_sb
        b1_sb = const_pool.tile([10, 1], f32, tag="b1")
        nc.sync.dma_start(out=b1_sb[:], in_=b1_d.rearrange("(u o) -> u o", o=1))
        b2_sb = const_pool.tile([1, 1], f32, tag="b2")
        nc.sync.dma_start(out=b2_sb[:], in_=b2_d.rearrange("(u o) -> u o", o=1))

        # ---- s_contrib = W1s.T @ s_prev.T -> [10, BC] ----
        sT_sb = const_pool.tile([128, NC2 * BC], sdt, tag="sT")
        for c in range(NC2):
            sT_ps = ps_sT.tile([128, BC], f32, tag="sT")
            nc.tensor.matmul(
                sT_ps[:],
                lhsT=s_sb[:, 128 * c : 128 * (c + 1)],
                rhs=ident8[:],
                start=True,
                stop=True,
            )
            nc.vector.tensor_copy(sT_sb[:, BC * c : BC * (c + 1)], sT_ps[:])
        sc_ps = ps_h.tile([10, BC], f32, tag="h")
        for c in range(NC2):
            nc.tensor.matmul(
                sc_ps[:],
                lhsT=(w1_t if mm2_bf else w1_sb)[
                    :, 10 * (NC1 + c) : 10 * (NC1 + c) + 10
                ],
                rhs=sT_sb[:, BC * c : BC * (c + 1)],
                start=(c == 0),
                stop=(c == NC2 - 1),
            )
        bias_sb = const_pool.tile([10, BC], f32, tag="bias")
        nc.scalar.activation(bias_sb[:], sc_ps[:], AF.Identity, bias=b1_sb[:])

        # ---- per-batch main loop ----
        for b in range(BC):
            if mm2_bf:
                # loaded (with inline fp32->bf16 cast) up front
                src_tiles = [all_src[(b, tb)] for tb in range(NTB)]
            else:
                src_tiles = []
                a_tiles = []
                for tb in range(NTB):
                    a_t = a_pool.tile([128, D], f32, tag="a")
                    nc.sync.dma_start(
                        out=a_t[:], in_=a_d[b, 128 * tb : 128 * (tb + 1), :]
                    )
                    a_tiles.append(a_t)
                    if mm1_bf:
                        a_bf = abf_pool.tile([128, D], bf16, tag="abf")
                        nc.vector.tensor_copy(a_bf[:], a_t[:])
                        src_tiles.append(a_bf)
                    else:
                        src_tiles.append(a_t)
            mm2_tiles = src_tiles if mm2_bf else a_tiles

            # mm1: hT[10, T] = sum_c W1a_c.T @ aT_c
            # aT tiles come from PE matmul-transposes (even chunks) and
            # DMA xbar transposes (odd chunks, bf16 SBUF->SBUF) in parallel
            h_ps = ps_h.tile([10, T], f32, tag="h")
            aT_sbs = []
            for c in range(NC1):
                aT_sb = aT_pool.tile([128, T], tdt, tag="aT")
                aT_ps = ps_aT.tile([128, T], f32, tag="aT")
                for tb in range(NTB):
                    # one accumulation group across the 4 disjoint column
                    # slices of this bank — avoids a PSUM drain per block
                    nc.tensor.matmul(
                        aT_ps[:, 128 * tb : 128 * (tb + 1)],
                        lhsT=src_tiles[tb][:, 128 * c : 128 * (c + 1)],
                        rhs=ident_t[:],
                        start=(tb == 0),
                        stop=(tb == NTB - 1),
                        skip_group_check=True,
                    )
                if c % 2 == 0:
                    nc.vector.tensor_copy(aT_sb[:], aT_ps[:])
                else:
                    nc.scalar.copy(aT_sb[:], aT_ps[:])
                aT_sbs.append(aT_sb)
            # dense mm1 chain after the copies are in flight
            for c in range(NC1):
                nc.tensor.matmul(
                    h_ps[:],
                    lhsT=w1_t[:, 10 * c : 10 * (c + 1)],
                    rhs=aT_sbs[c][:],
                    start=(c == 0),
                    stop=(c == NC1 - 1),
                )

            # tanh(h + s_contrib[:, b] + b1)
            hT_sb = row_pool.tile([10, T], bf16 if mm2_bf else f32, tag="hT")
            nc.scalar.activation(
                hT_sb[:], h_ps[:], AF.Tanh, bias=bias_sb[:, b : b + 1]
            )

            # e = relu(W2.T @ hT + b2); exp + sum; normalize
            e_ps = ps_e.tile([1, T], f32, tag="e")
            nc.tensor.matmul(
                e_ps[:], lhsT=w2_t[:], rhs=hT_sb[:], start=True, stop=True
            )
            # exp(relu(x + b2)) == max(exp(x + b2), 1); fold the relu into a
            # DVE max that also produces the softmax denominator
            ee_sb = row_pool.tile([1, T], f32, tag="ee")
            nc.scalar.activation(ee_sb[:], e_ps[:], AF.Exp, bias=b2_sb[:])
            ex_sb = row_pool.tile([1, T], f32, tag="ex")
            sum_sb = row_pool.tile([1, 1], f32, tag="sum")
            nc.vector.tensor_scalar(
                ex_sb[:],
                ee_sb[:],
                1.0,
                None,
                op0=mybir.AluOpType.max,
                op1=mybir.AluOpType.add,
                accum_out=sum_sb[:],
            )
            rec_sb = row_pool.tile([1, 1], f32, tag="rec")
            nc.vector.reciprocal(rec_sb[:], sum_sb[:])
            if mm2_bf:
                sco_t = row_pool.tile([1, T], bf16, tag="sco_bf")
                nc.vector.tensor_scalar_mul(sco_t[:], ex_sb[:], rec_sb[:])
            sco_sb = row_pool.tile([1, T], f32, tag="sco")
            nc.vector.tensor_scalar_mul(sco_sb[:], ex_sb[:], rec_sb[:])
            nc.sync.dma_start(out=sco_d[b : b + 1, :], in_=sco_sb[:])
            if not mm2_bf:
                sco_t = sco_sb

            # transpose scores -> [128, NTB] via K=1 matmuls
            scT_ps = ps_sT.tile([128, NTB], f32, tag="sT")
            for tb in range(NTB):
                nc.tensor.matmul(
                    scT_ps[:, tb : tb + 1],
                    lhsT=sco_t[:, 128 * tb : 128 * (tb + 1)],
                    rhs=ones11[:],
                    start=(tb == 0),
                    stop=(tb == NTB - 1),
                    skip_group_check=True,
                )
            scT_sb = row_pool.tile([128, NTB], sdt, tag="scT")
            nc.vector.tensor_copy(scT_sb[:], scT_ps[:])

            # mm2: ctx[1, D] = sum_tb scT_tb.T @ a_tb
            # tb-outer so each score-column weight load serves both halves
            ctx_ps = ps_ctx.tile([1, D], f32, tag="ctx")
            for tb in range(NTB):
                for hd in range(2):
                    nc.tensor.matmul(
                        ctx_ps[:, 512 * hd : 512 * (hd + 1)],
                        lhsT=scT_sb[:, tb : tb + 1],
                        rhs=mm2_tiles[tb][:, 512 * hd : 512 * (hd + 1)],
                        start=(tb == 0),
                        stop=(tb == NTB - 1),
                        skip_group_check=True,
                    )
            ctx_sb = row_pool.tile([1, D], f32, tag="ctxr")
            nc.vector.tensor_copy(ctx_sb[:, 0:512], ctx_ps[:, 0:512])
            nc.scalar.copy(ctx_sb[:, 512:1024], ctx_ps[:, 512:1024])
            nc.sync.dma_start(out=ctx_d[b : b + 1, :], in_=ctx_sb[:])

    nc.compile()
    return nc


def get_nc(mode=MODE):
    if mode not in _CACHE:
        _CACHE[mode] = _build(mode)
    return _CACHE[mode]


def make_in_maps(a, s_prev, W1, b1, W2, b2):
    a = np.ascontiguousarray(np.asarray(a, dtype=np.float32))
    s_prev = np.ascontiguousarray(np.asarray(s_prev, dtype=np.float32))
    W1 = np.ascontiguousarray(np.asarray(W1, dtype=np.float32))
    b1 = np.ascontiguousarray(np.asarray(b1, dtype=np.float32))
    W2 = np.ascontiguousarray(np.asarray(W2, dtype=np.float32))
    b2 = np.ascontiguousarray(np.asarray(b2, dtype=np.float32))
    in_maps = []
    for i in range(NCORES):
        sl = slice(i * BC, (i + 1) * BC)
        in_maps.append(
            {
                "a": a[sl],
                "s_prev": s_prev[sl],
                "W1": W1,
                "b1": b1,
                "W2": W2,
                "b2": b2,
            }
        )
    return in_maps


def assemble(results):
    ctx = np.concatenate([r["ctx_out"] for r in results], axis=0)  # [B, D]
    sco = np.concatenate([r["score_out"] for r in results], axis=0)  # [B, T]
    context_vector = ctx.reshape(B, 1, D).astype(np.float32)
    attention_score = sco.reshape(B, T, 1).astype(np.float32)
    return context_vector, attention_score


def run_spmd(inputs, trace=False, mode=MODE, **kwargs):
    from concourse.bass_utils import run_bass_kernel_spmd

    nc = get_nc(mode)
    in_maps = make_in_maps(**inputs)
    res = run_bass_kernel_spmd(nc, in_maps, list(range(NCORES)), trace=trace, **kwargs)
    return res


def kernel(a, s_prev, W1, b1, W2, b2):
    res = run_spmd(dict(a=a, s_prev=s_prev, W1=W1, b1=b1, W2=W2, b2=b2))
    return assemble(res.results)
